# revision 13
# baseline (speedup 1.0000x reference)
"""Multi-head causal self-attention on 8 Trainium2 NeuronCores (v3).

Problem: B=2, T=2048, C=1024, H=16 heads, D=64 head_dim.
  qkv = x @ Wqkv; causal softmax attention per head; y = attn_out @ Wout.

Sharding (2-way data parallel on batch x 4-way tensor parallel on heads):
  core c -> batch b = c // 4, head group g = c % 4 (heads 4g..4g+3).
  Host sums the 4 partial out-projections per batch (the TP all-reduce).

v3: one uniform software pipeline over global q-tiles g = rep*4 + qt.
The attention S/exp stream for tile g is ACT-bound (exp costs ~2.4x the
matmul time of the S tiles it consumes), so all PE-heavy work is emitted
as interleaved "fillers" inside it:
  stream(g) fillers = [out-projection of g-2] + [x-load/transpose/V-proj
  prep of g+1] + [QK projection of g+1] + [PV/normalize of g's first head
  pair during the second S stream].
Cross-rep: rep r+1's prep work fills rep r's deepest exp stream (qt=3),
so the steady-state per-rep time approaches the PE roofline.

Everything PSUM flows through 3 pools (8 banks): st (2x[128,2,512]; shared
by S tiles, QK projection, x-transposes, and V projection), pv
(2x[128,2,65]) and ops (1x[128,2,512]).

Numerics: all matmul operands bf16 (weights cast host-side, x cast during
transpose evacuation; PSUM f32). PV runs in natural orientation with the
softmax denominator riding as column 64 of [V|1], so normalization is a
per-partition reciprocal+broadcast-multiply. y is transposed back for the
out-projection via bf16 dma_start_transpose. exp folds the 1/sqrt(D)
scale. Causal masking: fully-masked column ranges of diagonal S tiles are
skipped (matmul and exp), the diagonal 128x128 triangle is masked
multiplicatively after exp.

build(repeat=N) emits N pipelined reps (test.py timing).
"""

import math
from contextlib import ExitStack

import numpy as np
import ml_dtypes

import concourse.bass as bass
import concourse.mybir as mybir
import concourse.tile as tile
from concourse import bacc
from concourse.bass_utils import run_bass_kernel_spmd
from concourse.masks import make_identity

B, T, C = 2, 2048, 1024
H, D = 16, 64
NCORES = 8
TPG = 4            # tensor-parallel group size (cores per batch)
HG = H // TPG      # heads per core = 4
HCOLS = HG * D     # 256 qkv columns per core
CB = C // 128      # 8 chan blocks
TB = T // 128      # 16 token blocks
QT = T // 512      # 4 q tiles
SCALE = 1.0 / math.sqrt(D)

F32 = mybir.dt.float32
F32R = mybir.dt.float32r
BF = mybir.dt.bfloat16

_CACHE = {}

# Risky-on-toolchain optimizations, individually verifiable:
YP_FROM_PSUM = False  # bass forbids DMA reads of PSUM; staging required
TRANSP_F32R = False   # f32r transpose path miscomputes on HW


def _build_body(nc, tc, repeat, tensors):
    Exp = mybir.ActivationFunctionType.Exp
    Copy = mybir.ActivationFunctionType.Copy
    NG = QT * repeat
    xb, wq, wk, wv, wo, yp = tensors

    with ExitStack() as pctx:
        ep = pctx.enter_context

        # ---- constants + cross-rep persistent tiles (written every rep,
        # ---- conflicts are always a full pipeline stage apart) ----
        pp = ep(tc.tile_pool(name="pp", bufs=1))
        xT = pp.tile([128, CB, T], BF, tag="xT")            # 16KB/part
        kT = pp.tile([128, 2, T], BF, tag="kT")
        qZ = [pp.tile([128, T], BF, tag=f"qZ{h}", name=f"qZ{h}")
              for h in range(HG)]
        yT = pp.tile([128, 2, T], BF, tag="yT")             # [pairdim,hp,tok]
        ident = pp.tile([128, 128], F32, tag="ident")
        tri = pp.tile([128, 128], BF, tag="tri")
        tri_f = pp.tile([128, 128], F32, tag="tri_f")
        ones_f = pp.tile([128, 1], F32, tag="ones_f")

        make_identity(nc, ident[:])
        if TRANSP_F32R:
            ident_r = pp.tile([128, 128], F32R, tag="ident_r")
            nc.vector.tensor_copy(ident_r[:], ident[:])
        nc.gpsimd.memset(ones_f[:], 1.0)
        for h in range(HG):
            dead = slice(64, 128) if h % 2 == 0 else slice(0, 64)
            nc.gpsimd.memset(qZ[h][dead, :].bitcast(F32), 0.0)
        nc.gpsimd.memset(tri_f[:], 1.0)
        nc.gpsimd.affine_select(
            out=tri_f[:], in_=tri_f[:], compare_op=mybir.AluOpType.is_ge,
            fill=0.0, base=0, pattern=[[1, 128]], channel_multiplier=-1)
        nc.vector.tensor_copy(tri[:], tri_f[:])

        # ---- pools ----
        st_ps = ep(tc.tile_pool(name="st_ps", bufs=2, space="PSUM"))
        pv_ps = ep(tc.tile_pool(name="pv_ps", bufs=2, space="PSUM"))
        op_ps = ep(tc.tile_pool(name="op_ps", bufs=2, space="PSUM"))
        wpool = ep(tc.tile_pool(name="wpool", bufs=1))
        v1_pool = ep(tc.tile_pool(name="v1p", bufs=2))
        pt_pool = ep(tc.tile_pool(name="pt", bufs=2))
        nrm_pool = ep(tc.tile_pool(name="nrm", bufs=4))
        out_pool = ep(tc.tile_pool(name="out", bufs=2))
        xn_pool = ep(tc.tile_pool(name="xn", bufs=4))
        yn_pool = ep(tc.tile_pool(name="yn", bufs=4))

        state = {}
        xn_tiles = {}

        def ensure_rep(r):
            if r in state:
                return state[r]
            s = {
                "wq": wpool.tile([128, CB, HCOLS], BF, tag="wq",
                                 name=f"wq{r}"),
                "wk": wpool.tile([128, CB, HCOLS], BF, tag="wk",
                                 name=f"wk{r}"),
                "wv": wpool.tile([128, CB, HCOLS], BF, tag="wv",
                                 name=f"wv{r}"),
                "wo": wpool.tile([128, 2, C], BF, tag="wo", bufs=2,
                                 name=f"wo{r}"),
                "v1": v1_pool.tile([128, TB, HG, 65], BF, tag="v1",
                                   name=f"v1_{r}"),
            }
            nc.sync.dma_start(s["wq"][:],
                              wq.rearrange("(cb p) n -> p cb n", p=128))
            nc.sync.dma_start(s["wk"][:],
                              wk.rearrange("(cb p) n -> p cb n", p=128))
            nc.sync.dma_start(s["wv"][:],
                              wv.rearrange("(cb p) n -> p cb n", p=128))
            nc.sync.dma_start(s["wo"][:],
                              wo.rearrange("(pb p) n -> p pb n", p=128))
            oap = ones_f[:, :]
            nc.vector.tensor_copy(
                s["v1"][:, :, :, 64],
                bass.AP(oap.tensor, oap.offset,
                        [oap.ap[0], [0, TB], [0, HG]]))
            state[r] = s
            return s

        def issue_x(g):
            """Issue all 4 x-row-block DMAs for tile g + rep-level loads."""
            tiles = []
            xdt = F32R if TRANSP_F32R else F32
            for tb4 in range(4):
                tb = 4 * (g % QT) + tb4
                xn = xn_pool.tile([128, C], xdt, tag="xn",
                                  name=f"xn{g}_{tb4}")
                nc.sync.dma_start(xn[:],
                                  xb[tb * 128:(tb + 1) * 128, :].bitcast(xdt))
                tiles.append(xn)
            xn_tiles[g] = tiles
            ensure_rep(g // QT)

        def prep_transpose(g, tb4):
            tb = 4 * (g % QT) + tb4
            xn = xn_tiles[g][tb4]
            xdt = F32R if TRANSP_F32R else F32
            idt = ident_r if TRANSP_F32R else ident
            for half in range(2):
                tp = op_ps.tile([128, 512], xdt, tag="ops", name="tpq")
                for j in range(4):
                    cb = 4 * half + j
                    nc.tensor.transpose(
                        tp[:, j * 128:(j + 1) * 128],
                        xn[:, cb * 128:(cb + 1) * 128], idt[:])
                dst = xT[:, 4 * half:4 * half + 4, tb * 128:(tb + 1) * 128]
                src = tp[:].rearrange("p (b t) -> p b t", t=128)
                nc.vector.tensor_copy(dst, src)

        def prep_v(g, tb4):
            r = g // QT
            s = ensure_rep(r)
            tb = 4 * (g % QT) + tb4
            vt = op_ps.tile([128, 512], F32, tag="ops", name="vq")
            vps = vt[:, 0:HCOLS]
            for cb in range(CB):
                nc.tensor.matmul(
                    vps, xT[:, cb, tb * 128:(tb + 1) * 128],
                    s["wv"][:, cb, :], start=(cb == 0), stop=(cb == CB - 1))
            nc.vector.tensor_copy(
                s["v1"][:, tb, :, 0:64],
                vps.rearrange("p (h d) -> p h d", d=64))

        def qkproj(g, m):
            r, qt = g // QT, g % QT
            s = ensure_rep(r)
            tsl = slice(qt * 512, (qt + 1) * 512)
            qk = st_ps.tile([128, 2, 512], F32, tag="st", name="qk")
            for cb in range(CB):
                nc.tensor.matmul(
                    qk[:, 0, :], s["wq"][:, cb, m * 128:(m + 1) * 128],
                    xT[:, cb, tsl], start=(cb == 0), stop=(cb == CB - 1))
            for cb in range(CB):
                nc.tensor.matmul(
                    qk[:, 1, :], s["wk"][:, cb, m * 128:(m + 1) * 128],
                    xT[:, cb, tsl], start=(cb == 0), stop=(cb == CB - 1))
            nc.vector.tensor_copy(qZ[2 * m][0:64, tsl], qk[0:64, 0, :])
            nc.vector.tensor_copy(qZ[2 * m + 1][64:128, tsl],
                                  qk[64:128, 0, :])
            nc.vector.tensor_copy(kT[:, m, tsl], qk[:, 1, :])

        def s_tile(g, hp, pt, hh, ki2):
            qt = g % QT
            h = 2 * hp + hh
            st2 = st_ps.tile([128, 2, 512], F32, tag="st", name="st2")
            offs = []
            for j in range(2):
                ki = ki2 + j
                jd = ki - 4 * qt
                off = 128 * jd if jd > 0 else 0
                offs.append(off)
                nc.tensor.matmul(
                    st2[:, j, off:512],
                    kT[:, hp, ki * 128:(ki + 1) * 128],
                    qZ[h][:, qt * 512 + off:(qt + 1) * 512],
                    start=True, stop=True)
            if offs == [0, 0]:
                nc.scalar.activation(pt[:, hh, ki2:ki2 + 2, :], st2[:], Exp,
                                     scale=SCALE)
            else:
                for j in range(2):
                    nc.scalar.activation(pt[:, hh, ki2 + j, offs[j]:512],
                                         st2[:, j, offs[j]:512], Exp,
                                         scale=SCALE)
            for j in range(2):
                ki = ki2 + j
                jd = ki - 4 * qt
                if jd >= 0:
                    sl = slice(128 * jd, 128 * jd + 128)
                    nc.gpsimd.tensor_mul(pt[:, hh, ki, sl], pt[:, hh, ki, sl],
                                         tri[:])

        def pv_norm(g, hp, pt, qb4):
            r, qt = g // QT, g % QT
            s = state[r]
            qb = 4 * qt + qb4
            pv = pv_ps.tile([128, 2, 65], F32, tag="pv")
            for hh in range(2):
                for ki in range(qb + 1):
                    nc.tensor.matmul(
                        pv[:, hh, :],
                        pt[:, hh, ki, qb4 * 128:(qb4 + 1) * 128],
                        s["v1"][:, ki, 2 * hp + hh, :],
                        start=(ki == 0), stop=(ki == qb))
            recip = nrm_pool.tile([128, 2], F32, tag="recip")
            nc.vector.reciprocal(recip[:], pv[:, :, 64])
            rap = recip[:, :]
            rb = bass.AP(rap.tensor, rap.offset, rap.ap + [[0, D]])
            ynt = yn_pool.tile([128, 2, D], BF, tag="yn", name="ynt")
            nc.vector.tensor_mul(ynt[:], pv[:, :, 0:D], rb)
            nc.sync.dma_start_transpose(
                yT[:, hp, qb * 128:(qb + 1) * 128], ynt[:])

        def outproj(g, tb4):
            r, qt = g // QT, g % QT
            s = state[r]
            tb = 4 * qt + tb4
            for ct in range(2):
                ops = op_ps.tile([128, 512], F32, tag="ops")
                for hp2 in range(2):
                    nc.tensor.matmul(
                        ops[:],
                        yT[:, hp2, tb * 128:(tb + 1) * 128],
                        s["wo"][:, hp2, ct * 512:(ct + 1) * 512],
                        start=(hp2 == 0), stop=(hp2 == 1))
                dst = yp[tb * 128:(tb + 1) * 128, ct * 512:(ct + 1) * 512]
                osb = out_pool.tile([128, 512], F32, tag="osb", name="osb")
                nc.vector.tensor_copy(osb[:], ops[:])
                nc.sync.dma_start(dst, osb[:])

        def prep_fillers(g):
            """PE filler items computing x-transposes, V and QK projection
            for global tile g (consumed inside stream g-1)."""
            if g >= NG:
                return []
            issue_x(g)
            # V projection lags its transpose by one block so the PSUM->SBUF
            # evacuation (RAW for the V matmuls) is hidden behind PE work.
            items = [(640, lambda: prep_transpose(g, 0)),
                     (640, lambda: prep_transpose(g, 1))]
            for tb4 in range(2, 4):
                items.append((853, lambda tb4=tb4: prep_v(g, tb4 - 2)))
                items.append((640, lambda tb4=tb4: prep_transpose(g, tb4)))
            items.append((853, lambda: prep_v(g, 2)))
            items.append((853, lambda: prep_v(g, 3)))
            return items

        def x_prep_full(g):
            """Non-filler variant: full prep of tile g's token blocks."""
            issue_x(g)
            for tb4 in range(4):
                prep_transpose(g, tb4)
                prep_v(g, tb4)

        S_SURPLUS = 611   # exp time minus matmul time per S tile (ns)

        def interleave(s_items, fillers, forced=None):
            """Emit S tiles, popping (cost, fn) fillers whenever the
            accumulated exp-over-matmul surplus covers them. `forced` maps
            1-based S-item indices to closures emitted exactly there (used
            for cross-rep QK projections with WAR ordering constraints)."""
            deficit = 0.0
            for i, it in enumerate(s_items, 1):
                it()
                deficit += S_SURPLUS
                if forced and i in forced:
                    forced.pop(i)()
                    deficit -= 3400
                while deficit > 0 and fillers:
                    c, f = fillers.pop(0)
                    f()
                    deficit -= c
            if forced:
                for i in sorted(forced):
                    forced.pop(i)()
            return fillers

        # ---- ramp: first tile's inputs ----
        x_prep_full(0)
        qkproj(0, 0)
        qkproj(0, 1)

        pending_pv1 = []   # PV(hp1) of the previous tile, filled into A
        pending_out = {}   # host stream -> out-projection filler items
        for g in range(NG):
            qt = g % QT
            nki = 4 * (qt + 1)
            # the cross-rep QK projection (boundary) writes kT/qZ slices
            # whose previous-rep readers are early stream-B S tiles, so it
            # is pinned late in stream B instead of floating in the queue
            boundary = (g + 1 < NG) and ((g + 1) % QT == 0 or qt == QT - 1)
            # out-projections of qt 0..2 all host in the next qt3 stream
            # (the deepest exp stream needs the most PE filler); qt3's host
            # two tiles later (its yT finishes in stream qt0's A)
            host = g + (3 - qt) if qt < QT - 1 else g + 2
            pending_out.setdefault(host, [])
            pending_out[host] += [(853, lambda tb4=tb4, g2=g: outproj(g2, tb4))
                                  for tb4 in range(4)]
            fill = list(pending_pv1)
            fill += pending_out.pop(g, [])
            fill += prep_fillers(g + 1)
            forced = None
            if g + 1 < NG:
                qk0 = lambda: qkproj(g + 1, 0)
                qk1 = lambda: qkproj(g + 1, 1)
                if boundary:
                    forced = {max(1, nki - 4): qk0, max(2, nki - 2): qk1}
                else:
                    fill += [(3400, qk0), (3400, qk1)]
            pt0 = pt_pool.tile([128, 2, 16, 512], BF, tag="pt", name="pt0")
            items = [lambda hh=hh, ki2=ki2: s_tile(g, 0, pt0, hh, ki2)
                     for hh in range(2) for ki2 in range(0, nki, 2)]
            fill = interleave(items, fill)
            pt1 = pt_pool.tile([128, 2, 16, 512], BF, tag="pt", name="pt1")
            fill += [(54 * (4 * qt + qb4 + 1),
                      lambda qb4=qb4: pv_norm(g, 0, pt0, qb4))
                     for qb4 in range(4)]
            items = [lambda hh=hh, ki2=ki2: s_tile(g, 1, pt1, hh, ki2)
                     for hh in range(2) for ki2 in range(0, nki, 2)]
            fill = interleave(items, fill, forced)
            for _, f in fill:
                f()
            pending_pv1 = [(54 * (4 * qt + qb4 + 1),
                            lambda qb4=qb4, g1=g, p1=pt1: pv_norm(g1, 1, p1,
                                                                  qb4))
                           for qb4 in range(4)]
        # tail: last tile's second head pair + out-projections without a
        # host stream
        for _, f in pending_pv1:
            f()
        for h in sorted(pending_out):
            for _, f in pending_out[h]:
                f()


def build(repeat=1):
    nc = bacc.Bacc("TRN2", target_bir_lowering=False, debug=False,
                   enable_asserts=False, num_devices=NCORES)
    tensors = (
        nc.dram_tensor("xb", [T, C], F32, kind="ExternalInput").ap(),
        nc.dram_tensor("wq", [C, HCOLS], BF, kind="ExternalInput").ap(),
        nc.dram_tensor("wk", [C, HCOLS], BF, kind="ExternalInput").ap(),
        nc.dram_tensor("wv", [C, HCOLS], BF, kind="ExternalInput").ap(),
        nc.dram_tensor("wo", [HCOLS, C], BF, kind="ExternalInput").ap(),
        nc.dram_tensor("yp", [T, C], F32, kind="ExternalOutput").ap(),
    )
    with tile.TileContext(nc) as tc:
        _build_body(nc, tc, repeat, tensors)

    nc.compile()
    return nc


def make_in_maps(x, Wqkv, Wout):
    x = np.ascontiguousarray(np.asarray(x), dtype=np.float32)
    Wqkv = np.asarray(Wqkv).astype(ml_dtypes.bfloat16)
    Wout = np.asarray(Wout).astype(ml_dtypes.bfloat16)
    in_maps = []
    for c in range(NCORES):
        b, g = c // TPG, c % TPG
        lo, hi = g * HCOLS, (g + 1) * HCOLS
        in_maps.append({
            "xb": x[b],
            "wq": np.ascontiguousarray(Wqkv[:, lo:hi]),
            "wk": np.ascontiguousarray(Wqkv[:, C + lo:C + hi]),
            "wv": np.ascontiguousarray(Wqkv[:, 2 * C + lo:2 * C + hi]),
            "wo": np.ascontiguousarray(Wout[lo:hi, :]),
        })
    return in_maps


def combine_results(results):
    out = np.empty((B, T, C), dtype=np.float32)
    for b in range(B):
        out[b] = results[b * TPG]["yp"]
        for i in range(1, TPG):
            out[b] += results[b * TPG + i]["yp"]
    return out


def get_nc():
    if "nc" not in _CACHE:
        _CACHE["nc"] = build()
    return _CACHE["nc"]


def kernel(x, attn_mask, Wqkv, Wout):
    """Full inputs in, full output out. attn_mask is the causal tril mask
    (encoded in the kernel structure)."""
    x = np.asarray(x)
    assert x.shape == (B, T, C), x.shape
    assert np.asarray(Wqkv).shape == (C, 3 * C)
    assert np.asarray(Wout).shape == (C, C)
    nc = get_nc()
    in_maps = make_in_maps(x, Wqkv, Wout)
    res = run_bass_kernel_spmd(nc, in_maps, core_ids=list(range(NCORES)))
    return combine_results(res.results)
